# revision 22
# baseline (speedup 1.0000x reference)
"""Single-head causal attention with RoPE + padding mask, data-parallel
over batch across 8 TRN2 NeuronCores (one batch element per core).

Per core (T=4096, C=128, HS=64):
  q = rope(x @ Wq); k = rope(x @ Wk); v = x @ Wv
  S^T[j,i] = k[j]·q[i]  (+ -1e30 on the strict upper triangle of the
        128x128 diagonal blocks, added by an identity-stationary
        accumulate-matmul -- no per-element select op needed)
  P^T = exp(S^T/sqrt(C))           (masked entries exp to exactly 0)
  outT[d,i] = sum_j (mask[j]*v[j,d]) P^T[j,i]; rowsum via a mask column
        appended to v (padding mask applied on the v/rowsum side)
  final out[i,d] = outT[d,i] / rowsum[i]  -- computed on HOST (the device
        returns outT [65, T]; host divides+transposes)

Performance structure (ScalarE exp is the bottleneck; everything else is
scheduled around keeping it dense):
  - 48 activations total: groups of 3 full j-tiles (FD=1536) plus ONE
    packed diagonal group per chunk (FD=1408, causal-shrunk, gap-free).
  - Causal masking fused into the score matmuls (tri-MM accumulate), so
    exp's dependencies are TensorE-only -> no multi-sem wait splitting on
    the Scalar queue.
  - Projections use column-duplicated stationaries [W|W] so rope q/k come
    out of the matmul already duplicated into partitions 64-127 for the
    row-packed score pairs.
  - v for ALL chunks is computed at the head through the (then idle) sg
    PSUM banks, right after a short burner-matmul warmup that brings the
    PE out of its HAM half-clock state.
  - Input DMAs are spread across the sync/gpsimd/scalar queues so chunk-0
    data lands early; software-pipelined emission keeps PV matmuls one
    group behind the score matmuls.
  - PSUM: sg [128,1536] x2 (6 banks) + proj [128,512] x1 + outT x1.
"""

import numpy as np

T, C, HS = 4096, 128, 64
N_CORES = 8
NT = T // 128      # 32 j-tiles of 128
NCH = T // 512     # 8 i-chunks of 512
SCALE = float(1.0 / np.sqrt(np.float32(C)))

_CACHE = {}


def _install_tile_drain_patch(tile_mod):
    """This container's walrus rejects instructions with >2 sem waits; split
    Tile's final global drain into one drain per ticked processor."""
    import bass_rust
    from concourse.vector_clock import ScopedClock

    def _patched(self, tick_clock, wait_clock):
        gc = tick_clock.global_clock
        for i in range(len(gc)):
            if gc[i] <= 0:
                continue
            v = bass_rust.VectorClock()
            v.require_at_least(i, gc[i])
            d = self.nc.sync.drain()
            wait_clock.add_sem_waits(d.ins, ScopedClock({None: v}))
        self.nc.all_engine_barrier()
        assert self.sems is not None
        popped = self.nc._tile_sem_poison_stack.pop()
        assert popped is self._sem_poison
        self.nc.clear_and_free_semaphores(list(self.sems.allocated().values()))
        self.nc.all_engine_barrier()

    tile_mod.TileContext._drain_and_barrier = _patched


def _split_excess_waits(nc, mybir, limit=1):
    """This container's walrus rejects instructions with >limit sem waits.
    Hoist excess waits onto standalone EventSemaphore instructions inserted
    just before the offending instruction on the same engine queue."""
    ctr = 0
    for f in nc.m.functions:
        for b in f.blocks:
            il = b.instructions
            out = []
            changed = False
            for ins in il:
                si = ins.sync_info
                waits = list(si.on_wait) if si and si.on_wait else []
                if len(waits) > limit:
                    changed = True
                    excess = waits[: len(waits) - limit]
                    keep = waits[len(waits) - limit :]
                    for i in range(0, len(excess), limit):
                        chunk = excess[i : i + limit]
                        ev = mybir.InstEventSemaphore(
                            name=f"I-waitsplit-{ctr}",
                            engine=ins.engine,
                            ins=[],
                            outs=[],
                            sync_info=mybir.SyncInfo(on_wait=chunk, on_update=[]),
                        )
                        ctr += 1
                        nc.register_instruction(ev)
                        out.append(ev)
                    si.on_wait = keep
                out.append(ins)
            if changed:
                b.instructions = out
    return nc


def _groups_for_chunk(ic):
    """Group layout for i-chunk ic.  Each group is a list of entries
    (jt, i_lo, sg_off, width, diag_off, ro):
      jt      -- j-tile index (of 128 rows)
      i_lo    -- first i column (within the 512-wide chunk) this tile needs
      sg_off  -- column offset in the score/pt tile where it is packed
      width   -- number of columns (512 - i_lo)
      diag_off-- sg_off of the 128-wide true-diagonal block needing the
                 -1e30 upper-triangle add, or None
      ro      -- PE row group (tile_position); entries with distinct ro run
                 concurrently and must then target distinct PSUM banks
    Non-diagonal j-tiles in groups of 3 (FD=1536); the 4 diagonal-band
    tiles causal-shrunk and packed gap-free into ONE group (FD=1280):
    t0:[0:512], t1:[512:896], t3:[896:1024], t2:[1024:1280] with matmul
    issue order (t0 | t1), t3-after-t1 (same row group, same bank),
    t2-after-t0, keeping concurrent pairs in distinct banks.
    """
    groups = []
    n = 4 * ic
    p = 0
    while p < n:
        take = 3 if (n - p) != 4 else 2
        take = min(take, n - p)
        ent = []
        for idx in range(take):
            ent.append((p + idx, 0, idx * 512, 512, None, 64 * (idx % 2)))
        groups.append(ent)
        p += take
    b = 4 * ic
    groups.append(
        [
            (b + 0, 0, 0, 512, 0, 0),
            (b + 1, 128, 512, 384, 512, 64),
            (b + 3, 384, 896, 128, 896, 64),
            (b + 2, 256, 1024, 256, 1024, 0),
        ]
    )
    return groups


def _build_nc():
    import concourse.bass as bass
    import concourse.mybir as mybir
    from concourse import tile

    _install_tile_drain_patch(tile)

    DT = mybir.dt
    F32, BF16 = DT.float32, DT.bfloat16
    AF = mybir.ActivationFunctionType

    nc = bass.Bass()
    xT_e = nc.declare_dram_parameter("xT", [C, T], BF16, isOutput=False)
    # w packed: [C, 576] = [Wq|Wq](128), [Wq_sw|Wq_sw](128), [Wk|Wk](128),
    #                      [Wk_sw|Wk_sw](128), Wv(64)
    w_e = nc.declare_dram_parameter("w", [C, 576], BF16, isOutput=False)
    cosx_e = nc.declare_dram_parameter("cosx", [128, T], BF16, isOutput=False)
    sinx_e = nc.declare_dram_parameter("sinx", [128, T], BF16, isOutput=False)
    mask01_e = nc.declare_dram_parameter("mask01", [128, NT], F32, isOutput=False)
    # tri: [identity | U_neg] where U_neg[j,i] = -1e30 if i<j else 0
    tri_e = nc.declare_dram_parameter("tri", [128, 256], BF16, isOutput=False)
    # out: [65, T] fp32; rows 0-63 = outT (d-major), row 64 = rowsum.
    out_e = nc.declare_dram_parameter("out", [HS + 1, T], F32, isOutput=True)

    with tile.TileContext(nc) as tc:
        with (
            tc.tile_pool(name="const", bufs=1) as cpool,
            tc.tile_pool(name="work", bufs=3) as wpool,
            tc.tile_pool(name="ps", bufs=2, space="PSUM") as ps,
        ):
            xT = cpool.tile([C, T], BF16)
            w_sb = cpool.tile([C, 576], BF16)
            mask01 = cpool.tile([128, NT], F32)
            tri_sb = cpool.tile([128, 256], BF16)
            cosx = cpool.tile([128, T], BF16)
            sinx = cpool.tile([128, T], BF16)

            def _sl(ch):
                return slice(ch * 512, (ch + 1) * 512)

            # input DMAs across three queues; gpsimd streams all xT chunks
            # first (v for every chunk is computed at the head), scalar only
            # carries head-time loads (its ACT stream starts ~15us in)
            nc.sync.dma_start(out=w_sb[:, :], in_=w_e[:, :])
            nc.gpsimd.dma_start(out=mask01[:, :], in_=mask01_e[:, :])
            nc.sync.dma_start(out=tri_sb[:, :], in_=tri_e[:, :])
            for ch in range(NCH):
                nc.gpsimd.dma_start(out=xT[:, _sl(ch)], in_=xT_e[:, _sl(ch)])
            nc.scalar.dma_start(out=cosx[:, _sl(0)], in_=cosx_e[:, _sl(0)])
            nc.sync.dma_start(out=sinx[:, _sl(0)], in_=sinx_e[:, _sl(0)])
            nc.scalar.dma_start(out=cosx[:, _sl(1)], in_=cosx_e[:, _sl(1)])
            nc.sync.dma_start(out=sinx[:, _sl(1)], in_=sinx_e[:, _sl(1)])
            for ch in range(2, NCH):
                nc.sync.dma_start(out=cosx[:, _sl(ch)], in_=cosx_e[:, _sl(ch)])
                if ch < 5:
                    nc.sync.dma_start(out=sinx[:, _sl(ch)], in_=sinx_e[:, _sl(ch)])
                else:
                    nc.gpsimd.dma_start(out=sinx[:, _sl(ch)], in_=sinx_e[:, _sl(ch)])

            # q2/k2: rows 0..63 = rope(q/k)^T, rows 64..127 identical copy
            q2 = cpool.tile([128, T], BF16)
            k2 = cpool.tile([128, T], BF16)

            # v tiles + mask column (mask-weighted rowsum): [t, j_tile, 65]
            vplus = cpool.tile([128, NT, HS + 1], BF16)
            nc.vector.tensor_copy(vplus[:, :, HS], mask01[:, :])

            # HAM warm-up burners: PE busy from ~7.5us so it reaches full
            # clock before the real dependency-paced pipeline ramps up
            scratch = cpool.tile([128, 512], BF16)
            nc.vector.memset(scratch[:, :], 0.0)
            burn = ps.tile([128, 1536], F32, tag="sg", bufs=2, name="burn")
            for r in range(9):
                nc.tensor.matmul(
                    burn[:, 0:512], scratch[:, 0:128], scratch[:, :],
                    start=True, stop=True,
                )

            def proj_q(ch, t_raw, o_raw, t_swp, o_swp):
                sl = _sl(ch)
                nc.tensor.matmul(
                    t_raw[:, o_raw : o_raw + 512], w_sb[:, 0:128], xT[:, sl],
                    start=True, stop=True,
                )
                nc.tensor.matmul(
                    t_swp[:, o_swp : o_swp + 512], w_sb[:, 128:256], xT[:, sl],
                    start=True, stop=True,
                )
                m1 = wpool.tile([128, 512], BF16, tag="rope", bufs=4, name=f"m1_{ch}")
                nc.vector.tensor_mul(m1[:, :], t_raw[:, o_raw : o_raw + 512], cosx[:, sl])
                m2 = wpool.tile([128, 512], BF16, tag="rope", bufs=4, name=f"m2_{ch}")
                nc.vector.tensor_mul(m2[:, :], t_swp[:, o_swp : o_swp + 512], sinx[:, sl])
                nc.vector.tensor_add(q2[:, sl], m1[:, :], m2[:, :])

            def proj_k(ch, t_raw, o_raw, t_swp, o_swp):
                sl = _sl(ch)
                nc.tensor.matmul(
                    t_raw[:, o_raw : o_raw + 512], w_sb[:, 256:384], xT[:, sl],
                    start=True, stop=True,
                )
                nc.tensor.matmul(
                    t_swp[:, o_swp : o_swp + 512], w_sb[:, 384:512], xT[:, sl],
                    start=True, stop=True,
                )
                m3 = wpool.tile([128, 512], BF16, tag="rope", bufs=4, name=f"m3_{ch}")
                nc.vector.tensor_mul(m3[:, :], t_raw[:, o_raw : o_raw + 512], cosx[:, sl])
                m4 = wpool.tile([128, 512], BF16, tag="rope", bufs=4, name=f"m4_{ch}")
                nc.vector.tensor_mul(m4[:, :], t_swp[:, o_swp : o_swp + 512], sinx[:, sl])
                nc.vector.tensor_add(k2[:, sl], m3[:, :], m4[:, :])

            def v_tiles(tile_ps, jts, base):
                for n, jt in enumerate(jts):
                    o = base + n * HS
                    nc.tensor.matmul(
                        tile_ps[:, o : o + HS],
                        xT[:, jt * 128 : (jt + 1) * 128],
                        w_sb[:, 512:576],
                        start=True,
                        stop=True,
                    )
                for n, jt in enumerate(jts):
                    o = base + n * HS
                    nc.vector.tensor_scalar_mul(
                        vplus[:, jt, 0:HS], tile_ps[:, o : o + HS],
                        mask01[:, jt : jt + 1],
                    )

            # head projections for chunks 0,1 + all v through the (idle) sg
            # banks so the 4 matmuls of each rope pair run bank-parallel
            hb1 = ps.tile([128, 1536], F32, tag="sg", bufs=2, name="hb1")
            proj_q(0, hb1, 0, hb1, 512)
            hb2 = ps.tile([128, 1536], F32, tag="sg", bufs=2, name="hb2")
            proj_k(0, hb2, 0, hb2, 512)
            v_tiles(hb2, range(0, 8), 1024)
            hb3 = ps.tile([128, 1536], F32, tag="sg", bufs=2, name="hb3")
            proj_q(1, hb3, 0, hb3, 512)
            v_tiles(hb3, range(8, 16), 1024)
            hb4 = ps.tile([128, 1536], F32, tag="sg", bufs=2, name="hb4")
            proj_k(1, hb4, 0, hb4, 512)
            v_tiles(hb4, range(16, 24), 1024)
            hb5 = ps.tile([128, 1536], F32, tag="sg", bufs=2, name="hb5")
            v_tiles(hb5, range(24, 32), 0)

            def proj_chunk(ch):
                # chunks 2..7: single dedicated PSUM bank, chained via DVE
                pr = ps.tile([128, 512], F32, tag="proj", bufs=1, name=f"pq{ch}")
                proj_q(ch, pr, 0, pr, 0)
                pr2 = ps.tile([128, 512], F32, tag="proj", bufs=1, name=f"pk{ch}")
                proj_k(ch, pr2, 0, pr2, 0)

            # NOTE: proj_q with t_raw == t_swp (same bank) relies on Tile
            # serializing the second matmul behind the m1 read -- intended.

            work = []  # (ic, group, is_last_of_chunk, bg list)
            for ic in range(NCH):
                gs = _groups_for_chunk(ic)
                for gi, g in enumerate(gs):
                    bg = []
                    if ic + 2 < NCH:
                        if gi == 0:
                            bg.append(ic + 2)
                    work.append((ic, g, gi == len(gs) - 1, bg))

            def emit_scores(ic, g, sg):
                # PSUM accumulation groups are bank-scoped: within each
                # 512-col bank, exactly one start=True (first score matmul
                # touching it) and one stop=True (the last tri matmul)
                isl0 = ic * 512
                diag = g[0][4] is not None
                seen = set()
                for jt, i_lo, off, w, _d, ro in g:
                    bank = off // 512
                    nc.tensor.matmul(
                        sg[:, off : off + w],
                        k2[ro : ro + HS, jt * 128 : (jt + 1) * 128],
                        q2[ro : ro + HS, isl0 + i_lo : isl0 + 512],
                        start=(bank not in seen),
                        stop=(not diag),
                        tile_position=(ro, 0),
                    )
                    seen.add(bank)
                if not diag:
                    return
                # fold the causal mask in: S[:, diag block] += -1e30 * U
                tri_banks = [e[4] // 512 for e in g]
                for n, (_jt, _i_lo, off, _w, d, _ro) in enumerate(g):
                    nc.tensor.matmul(
                        sg[:, d : d + 128],
                        tri_sb[:, 0:128],
                        tri_sb[:, 128:256],
                        start=False,
                        stop=(d // 512 not in tri_banks[n + 1 :]),
                    )

            def emit_pv(ic, g, pt, outT, last):
                for n, (jt, i_lo, off, w, _d, _ro) in enumerate(g):
                    nc.tensor.matmul(
                        outT[:, i_lo:512],
                        vplus[:, jt, :],
                        pt[:, off : off + w],
                        start=(jt == 0),
                        stop=(last and n == len(g) - 1),
                    )

            def emit_out(pic, pouT):
                osb = wpool.tile([HS + 1, 512], F32, tag="osb", bufs=2, name=f"osb{pic}")
                nc.vector.tensor_copy(osb[:, :], pouT[:, :])
                o0 = pic * 512
                nc.sync.dma_start(out=out_e[:, o0 : o0 + 256], in_=osb[:, 0:256])
                nc.gpsimd.dma_start(out=out_e[:, o0 + 256 : o0 + 512], in_=osb[:, 256:512])

            pending = None  # (ic, group, pt, outT, last?)
            outT_cur = None
            for ic, g, last, bg in work:
                fd = g[-1][2] + g[-1][3]
                if g[0][0] == 0:  # first group of chunk -> new outT bank
                    outT_cur = ps.tile(
                        [HS + 1, 512], F32, tag="outT", bufs=1, name=f"oT{ic}"
                    )
                sg = ps.tile([128, 1536], F32, tag="sg", bufs=2, name=f"sg{ic}_{g[0][0]}")
                emit_scores(ic, g, sg)
                if pending is not None:
                    pic, pg, ppt, pouT, plast = pending
                    emit_pv(pic, pg, ppt, pouT, plast)
                    if plast:
                        emit_out(pic, pouT)
                pt = wpool.tile(
                    [128, 1536], BF16, tag="pt", bufs=4, name=f"pt{ic}_{g[0][0]}"
                )
                nc.scalar.activation(
                    pt[:, 0:fd], sg[:, 0:fd], AF.Exp, bias=0.0, scale=SCALE
                )
                pending = (ic, g, pt, outT_cur, last)
                for cc in bg:
                    proj_chunk(cc)

            pic, pg, ppt, pouT, plast = pending
            emit_pv(pic, pg, ppt, pouT, plast)
            emit_out(pic, pouT)

    import concourse.mybir as mybir
    _split_excess_waits(nc, mybir, limit=1)
    return nc


def _get_nc():
    if "nc" not in _CACHE:
        _CACHE["nc"] = _build_nc()
    return _CACHE["nc"]



def kernel(x_text_emb, Wq, Wk, Wv, freqs_cos, freqs_sin, x_latex_mask):
    import ml_dtypes
    from concourse.bass_utils import run_bass_kernel_spmd

    bf16 = ml_dtypes.bfloat16
    nc = _get_nc()

    swap = np.arange(HS) ^ 1
    cos2 = np.repeat(np.asarray(freqs_cos, np.float32).T, 2, axis=0)  # [64, T]
    sin2s = np.repeat(np.asarray(freqs_sin, np.float32).T, 2, axis=0)
    sin2s[0::2] *= -1.0
    cosx = np.ascontiguousarray(np.tile(cos2, (2, 1))).astype(bf16)  # [128, T]
    sinx = np.ascontiguousarray(np.tile(sin2s, (2, 1))).astype(bf16)
    Wq = np.asarray(Wq, np.float32)
    Wk = np.asarray(Wk, np.float32)
    Wv = np.asarray(Wv, np.float32)
    w = np.concatenate(
        [Wq, Wq, Wq[:, swap], Wq[:, swap], Wk, Wk, Wk[:, swap], Wk[:, swap], Wv],
        axis=1,
    ).astype(bf16)
    w = np.ascontiguousarray(w)
    jj, ii = np.meshgrid(np.arange(128), np.arange(128), indexing="ij")
    tri = np.concatenate(
        [np.eye(128, dtype=np.float32), np.where(ii < jj, -1e30, 0.0)], axis=1
    ).astype(bf16)
    tri = np.ascontiguousarray(tri)
    # mask01[b] laid out [j_in_tile(128), j_tile(NT)]
    mask01 = np.asarray(x_latex_mask != 0, np.float32).reshape(N_CORES, NT, 128)

    in_maps = []
    for b in range(N_CORES):
        in_maps.append(
            {
                "xT": np.ascontiguousarray(
                    np.asarray(x_text_emb[b], np.float32).T
                ).astype(bf16),
                "w": w,
                "cosx": cosx,
                "sinx": sinx,
                "mask01": np.ascontiguousarray(mask01[b].T),
                "tri": tri,
            }
        )

    res = run_bass_kernel_spmd(nc, in_maps, core_ids=list(range(N_CORES)))
    # out arrives [65, T]: rows 0-63 = outT[d, i], row 64 = rowsum[i]
    outs = []
    for b in range(N_CORES):
        r = np.asarray(res.results[b]["out"], np.float32)
        outs.append((r[0:HS, :] / r[HS : HS + 1, :]).T)
    return np.stack(outs, axis=0)


# revision 24
# speedup vs baseline: 1.1892x; 1.1892x over previous
"""Single-head causal attention with RoPE + padding mask, data-parallel
over batch across 8 TRN2 NeuronCores (one batch element per core).

Per core (T=4096, C=128, HS=64):
  q = rope(x @ Wq); k = rope(x @ Wk); v = x @ Wv
  S^T[j,i] = k[j]·q[i]  (+ -1e30 on the strict upper triangle of the
        128x128 diagonal blocks, added by an identity-stationary
        accumulate-matmul -- no per-element select op needed)
  P^T = exp(S^T/sqrt(C))           (masked entries exp to exactly 0)
  outT[d,i] = sum_j (mask[j]*v[j,d]) P^T[j,i]; rowsum via a mask column
        appended to v (padding mask applied on the v/rowsum side)
  final out[i,d] = outT[d,i] / rowsum[i]  -- computed on HOST (the device
        returns outT [65, T]; host divides+transposes)

Performance structure (ScalarE exp is the bottleneck; everything else is
scheduled around keeping it dense):
  - 48 activations total: groups of 3 full j-tiles (FD=1536) plus ONE
    packed diagonal group per chunk (FD=1408, causal-shrunk, gap-free).
  - Causal masking fused into the score matmuls (tri-MM accumulate), so
    exp's dependencies are TensorE-only -> no multi-sem wait splitting on
    the Scalar queue.
  - Projections use column-duplicated stationaries [W|W] so rope q/k come
    out of the matmul already duplicated into partitions 64-127 for the
    row-packed score pairs.
  - v for ALL chunks is computed at the head through the (then idle) sg
    PSUM banks, right after a short burner-matmul warmup that brings the
    PE out of its HAM half-clock state.
  - Input DMAs are spread across the sync/gpsimd/scalar queues so chunk-0
    data lands early; software-pipelined emission keeps PV matmuls one
    group behind the score matmuls.
  - PSUM: sg [128,1536] x2 (6 banks) + proj [128,512] x1 + outT x1.
"""

import numpy as np

T, C, HS = 4096, 128, 64
N_CORES = 8
NT = T // 128      # 32 j-tiles of 128
NCH = T // 512     # 8 i-chunks of 512
SCALE = float(1.0 / np.sqrt(np.float32(C)))

_CACHE = {}


def _install_tile_drain_patch(tile_mod):
    """This container's walrus rejects instructions with >2 sem waits; split
    Tile's final global drain into one drain per ticked processor."""
    import bass_rust
    from concourse.vector_clock import ScopedClock

    def _patched(self, tick_clock, wait_clock):
        gc = tick_clock.global_clock
        for i in range(len(gc)):
            if gc[i] <= 0:
                continue
            v = bass_rust.VectorClock()
            v.require_at_least(i, gc[i])
            d = self.nc.sync.drain()
            wait_clock.add_sem_waits(d.ins, ScopedClock({None: v}))
        self.nc.all_engine_barrier()
        assert self.sems is not None
        popped = self.nc._tile_sem_poison_stack.pop()
        assert popped is self._sem_poison
        self.nc.clear_and_free_semaphores(list(self.sems.allocated().values()))
        self.nc.all_engine_barrier()

    tile_mod.TileContext._drain_and_barrier = _patched


def _split_excess_waits(nc, mybir, limit=1):
    """This container's walrus rejects instructions with >limit sem waits.
    Hoist excess waits onto standalone EventSemaphore instructions inserted
    just before the offending instruction on the same engine queue."""
    ctr = 0
    for f in nc.m.functions:
        for b in f.blocks:
            il = b.instructions
            out = []
            changed = False
            for ins in il:
                si = ins.sync_info
                waits = list(si.on_wait) if si and si.on_wait else []
                if len(waits) > limit:
                    changed = True
                    excess = waits[: len(waits) - limit]
                    keep = waits[len(waits) - limit :]
                    for i in range(0, len(excess), limit):
                        chunk = excess[i : i + limit]
                        ev = mybir.InstEventSemaphore(
                            name=f"I-waitsplit-{ctr}",
                            engine=ins.engine,
                            ins=[],
                            outs=[],
                            sync_info=mybir.SyncInfo(on_wait=chunk, on_update=[]),
                        )
                        ctr += 1
                        nc.register_instruction(ev)
                        out.append(ev)
                    si.on_wait = keep
                out.append(ins)
            if changed:
                b.instructions = out
    return nc


def _groups_for_chunk(ic):
    """Group layout for i-chunk ic.  Each group is a list of entries
    (jt, i_lo, sg_off, width, diag_off, ro):
      jt      -- j-tile index (of 128 rows)
      i_lo    -- first i column (within the 512-wide chunk) this tile needs
      sg_off  -- column offset in the score/pt tile where it is packed
      width   -- number of columns (512 - i_lo)
      diag_off-- sg_off of the 128-wide true-diagonal block needing the
                 -1e30 upper-triangle add, or None
      ro      -- PE row group (tile_position); entries with distinct ro run
                 concurrently and must then target distinct PSUM banks
    Non-diagonal j-tiles in groups of 3 (FD=1536); the 4 diagonal-band
    tiles causal-shrunk and packed gap-free into ONE group (FD=1280):
    t0:[0:512], t1:[512:896], t3:[896:1024], t2:[1024:1280] with matmul
    issue order (t0 | t1), t3-after-t1 (same row group, same bank),
    t2-after-t0, keeping concurrent pairs in distinct banks.
    """
    groups = []
    n = 4 * ic
    p = 0
    while p < n:
        take = 3 if (n - p) != 4 else 2
        take = min(take, n - p)
        ent = []
        for idx in range(take):
            ent.append((p + idx, 0, idx * 512, 512, None, 64 * (idx % 2)))
        groups.append(ent)
        p += take
    b = 4 * ic
    groups.append(
        [
            (b + 0, 0, 0, 512, 0, 0),
            (b + 1, 128, 512, 384, 512, 64),
            (b + 3, 384, 896, 128, 896, 64),
            (b + 2, 256, 1024, 256, 1024, 0),
        ]
    )
    return groups


def _build_nc():
    import concourse.bass as bass
    import concourse.mybir as mybir
    from concourse import tile

    _install_tile_drain_patch(tile)

    DT = mybir.dt
    F32, BF16 = DT.float32, DT.bfloat16
    AF = mybir.ActivationFunctionType

    nc = bass.Bass()
    xT_e = nc.declare_dram_parameter("xT", [C, T], BF16, isOutput=False)
    # w packed: [C, 576] = [Wq|Wq](128), [Wq_sw|Wq_sw](128), [Wk|Wk](128),
    #                      [Wk_sw|Wk_sw](128), Wv(64)
    w_e = nc.declare_dram_parameter("w", [C, 576], BF16, isOutput=False)
    cosx_e = nc.declare_dram_parameter("cosx", [128, T], BF16, isOutput=False)
    sinx_e = nc.declare_dram_parameter("sinx", [128, T], BF16, isOutput=False)
    mask01_e = nc.declare_dram_parameter("mask01", [128, NT], F32, isOutput=False)
    # tri: [identity | U_neg] where U_neg[j,i] = -1e30 if i<j else 0
    tri_e = nc.declare_dram_parameter("tri", [128, 256], BF16, isOutput=False)
    # out: [65, T] fp32; rows 0-63 = outT (d-major), row 64 = rowsum.
    out_e = nc.declare_dram_parameter("out", [HS + 1, T], F32, isOutput=True)

    with tile.TileContext(nc) as tc:
        with (
            tc.tile_pool(name="const", bufs=1) as cpool,
            tc.tile_pool(name="work", bufs=3) as wpool,
            tc.tile_pool(name="ps", bufs=2, space="PSUM") as ps,
        ):
            xT = cpool.tile([C, T], BF16)
            w_sb = cpool.tile([C, 576], BF16)
            mask01 = cpool.tile([128, NT], F32)
            tri_sb = cpool.tile([128, 256], BF16)
            cosx = cpool.tile([128, T], BF16)
            sinx = cpool.tile([128, T], BF16)

            def _sl(ch):
                return slice(ch * 512, (ch + 1) * 512)

            # input DMAs across three queues; gpsimd streams all xT chunks
            # first (v for every chunk is computed at the head), scalar only
            # carries head-time loads (its ACT stream starts ~15us in)
            nc.sync.dma_start(out=w_sb[:, :], in_=w_e[:, :])
            nc.gpsimd.dma_start(out=mask01[:, :], in_=mask01_e[:, :])
            nc.sync.dma_start(out=tri_sb[:, :], in_=tri_e[:, :])
            for ch in range(NCH):
                nc.gpsimd.dma_start(out=xT[:, _sl(ch)], in_=xT_e[:, _sl(ch)])
            nc.scalar.dma_start(out=cosx[:, _sl(0)], in_=cosx_e[:, _sl(0)])
            nc.sync.dma_start(out=sinx[:, _sl(0)], in_=sinx_e[:, _sl(0)])
            nc.scalar.dma_start(out=cosx[:, _sl(1)], in_=cosx_e[:, _sl(1)])
            nc.sync.dma_start(out=sinx[:, _sl(1)], in_=sinx_e[:, _sl(1)])
            for ch in range(2, NCH):
                nc.sync.dma_start(out=cosx[:, _sl(ch)], in_=cosx_e[:, _sl(ch)])
                if ch < 5:
                    nc.sync.dma_start(out=sinx[:, _sl(ch)], in_=sinx_e[:, _sl(ch)])
                else:
                    nc.gpsimd.dma_start(out=sinx[:, _sl(ch)], in_=sinx_e[:, _sl(ch)])

            # q2/k2: rows 0..63 = rope(q/k)^T, rows 64..127 identical copy
            q2 = cpool.tile([128, T], BF16)
            k2 = cpool.tile([128, T], BF16)

            # v tiles + mask column (mask-weighted rowsum): [t, j_tile, 65]
            vplus = cpool.tile([128, NT, HS + 1], BF16)
            nc.vector.tensor_copy(vplus[:, :, HS], mask01[:, :])

            # HAM warm-up burners: PE busy from ~7.5us so it reaches full
            # clock before the real dependency-paced pipeline ramps up
            scratch = cpool.tile([128, 512], BF16)
            nc.vector.memset(scratch[:, :], 0.0)
            burn = ps.tile([128, 1536], F32, tag="sg", bufs=2, name="burn")
            for r in range(9):
                nc.tensor.matmul(
                    burn[:, 0:512], scratch[:, 0:128], scratch[:, :],
                    start=True, stop=True,
                )

            def proj_q(ch, t_raw, o_raw, t_swp, o_swp):
                sl = _sl(ch)
                nc.tensor.matmul(
                    t_raw[:, o_raw : o_raw + 512], w_sb[:, 0:128], xT[:, sl],
                    start=True, stop=True,
                )
                nc.tensor.matmul(
                    t_swp[:, o_swp : o_swp + 512], w_sb[:, 128:256], xT[:, sl],
                    start=True, stop=True,
                )
                m1 = wpool.tile([128, 512], BF16, tag="rope", bufs=4, name=f"m1_{ch}")
                nc.vector.tensor_mul(m1[:, :], t_raw[:, o_raw : o_raw + 512], cosx[:, sl])
                m2 = wpool.tile([128, 512], BF16, tag="rope", bufs=4, name=f"m2_{ch}")
                nc.vector.tensor_mul(m2[:, :], t_swp[:, o_swp : o_swp + 512], sinx[:, sl])
                nc.vector.tensor_add(q2[:, sl], m1[:, :], m2[:, :])

            def proj_k(ch, t_raw, o_raw, t_swp, o_swp):
                sl = _sl(ch)
                nc.tensor.matmul(
                    t_raw[:, o_raw : o_raw + 512], w_sb[:, 256:384], xT[:, sl],
                    start=True, stop=True,
                )
                nc.tensor.matmul(
                    t_swp[:, o_swp : o_swp + 512], w_sb[:, 384:512], xT[:, sl],
                    start=True, stop=True,
                )
                m3 = wpool.tile([128, 512], BF16, tag="rope", bufs=4, name=f"m3_{ch}")
                nc.vector.tensor_mul(m3[:, :], t_raw[:, o_raw : o_raw + 512], cosx[:, sl])
                m4 = wpool.tile([128, 512], BF16, tag="rope", bufs=4, name=f"m4_{ch}")
                nc.vector.tensor_mul(m4[:, :], t_swp[:, o_swp : o_swp + 512], sinx[:, sl])
                nc.vector.tensor_add(k2[:, sl], m3[:, :], m4[:, :])

            def v_tiles(tile_ps, jts, base):
                for n, jt in enumerate(jts):
                    o = base + n * HS
                    nc.tensor.matmul(
                        tile_ps[:, o : o + HS],
                        xT[:, jt * 128 : (jt + 1) * 128],
                        w_sb[:, 512:576],
                        start=True,
                        stop=True,
                    )
                for n, jt in enumerate(jts):
                    o = base + n * HS
                    nc.vector.tensor_scalar_mul(
                        vplus[:, jt, 0:HS], tile_ps[:, o : o + HS],
                        mask01[:, jt : jt + 1],
                    )

            # head projections for chunks 0,1 + all v go through the (then
            # idle) sg banks so each rope pair's matmuls run bank-parallel;
            # they are interleaved with the first chunks' attention groups
            # (via head_pre below) so the first exp isn't gated on them all
            def hb_projq0():
                hb = ps.tile([128, 1536], F32, tag="sg", bufs=2, name="hb1")
                proj_q(0, hb, 0, hb, 512)

            def hb_projk0_v():
                hb = ps.tile([128, 1536], F32, tag="sg", bufs=2, name="hb2")
                proj_k(0, hb, 0, hb, 512)
                v_tiles(hb, range(0, 8), 1024)

            def hb_projq1_v():
                hb = ps.tile([128, 1536], F32, tag="sg", bufs=2, name="hb3")
                proj_q(1, hb, 0, hb, 512)
                v_tiles(hb, range(8, 16), 1024)

            def hb_projk1_v():
                hb = ps.tile([128, 1536], F32, tag="sg", bufs=2, name="hb4")
                proj_k(1, hb, 0, hb, 512)
                v_tiles(hb, range(16, 24), 1024)

            def hb_v_last():
                hb = ps.tile([128, 1536], F32, tag="sg", bufs=2, name="hb5")
                v_tiles(hb, range(24, 32), 0)

            head_pre = {
                (0, 0): [hb_projq0, hb_projk0_v],
                (1, 0): [hb_projq1_v],
                (1, 1): [hb_projk1_v],
                (1, 2): [hb_v_last],
            }

            def proj_chunk(ch):
                # chunks 2..7: single dedicated PSUM bank, chained via DVE
                pr = ps.tile([128, 512], F32, tag="proj", bufs=1, name=f"pq{ch}")
                proj_q(ch, pr, 0, pr, 0)
                pr2 = ps.tile([128, 512], F32, tag="proj", bufs=1, name=f"pk{ch}")
                proj_k(ch, pr2, 0, pr2, 0)

            # NOTE: proj_q with t_raw == t_swp (same bank) relies on Tile
            # serializing the second matmul behind the m1 read -- intended.

            work = []  # (ic, group, is_last_of_chunk, pre blocks, bg list)
            for ic in range(NCH):
                gs = _groups_for_chunk(ic)
                for gi, g in enumerate(gs):
                    bg = []
                    if ic + 2 < NCH and gi == 0:
                        bg.append(ic + 2)
                    pre = head_pre.get((ic, gi), [])
                    work.append((ic, g, gi == len(gs) - 1, pre, bg))

            def emit_scores(ic, g, sg):
                # PSUM accumulation groups are bank-scoped: within each
                # 512-col bank, exactly one start=True (first score matmul
                # touching it) and one stop=True (the last tri matmul)
                isl0 = ic * 512
                diag = g[0][4] is not None
                seen = set()
                for jt, i_lo, off, w, _d, ro in g:
                    bank = off // 512
                    nc.tensor.matmul(
                        sg[:, off : off + w],
                        k2[ro : ro + HS, jt * 128 : (jt + 1) * 128],
                        q2[ro : ro + HS, isl0 + i_lo : isl0 + 512],
                        start=(bank not in seen),
                        stop=(not diag),
                        tile_position=(ro, 0),
                    )
                    seen.add(bank)
                if not diag:
                    return
                # fold the causal mask in: S[:, diag block] += -1e30 * U
                tri_banks = [e[4] // 512 for e in g]
                for n, (_jt, _i_lo, off, _w, d, _ro) in enumerate(g):
                    nc.tensor.matmul(
                        sg[:, d : d + 128],
                        tri_sb[:, 0:128],
                        tri_sb[:, 128:256],
                        start=False,
                        stop=(d // 512 not in tri_banks[n + 1 :]),
                    )

            def emit_pv(ic, g, pt, outT, last):
                for n, (jt, i_lo, off, w, _d, _ro) in enumerate(g):
                    nc.tensor.matmul(
                        outT[:, i_lo:512],
                        vplus[:, jt, :],
                        pt[:, off : off + w],
                        start=(jt == 0),
                        stop=(last and n == len(g) - 1),
                    )

            def emit_out(pic, pouT):
                osb = wpool.tile([HS + 1, 512], F32, tag="osb", bufs=2, name=f"osb{pic}")
                nc.vector.tensor_copy(osb[:, :], pouT[:, :])
                o0 = pic * 512
                nc.sync.dma_start(out=out_e[:, o0 : o0 + 256], in_=osb[:, 0:256])
                nc.gpsimd.dma_start(out=out_e[:, o0 + 256 : o0 + 512], in_=osb[:, 256:512])

            pending = None  # (ic, group, pt, outT, last?)
            outT_cur = None
            for ic, g, last, pre, bg in work:
                for blk in pre:
                    blk()
                fd = g[-1][2] + g[-1][3]
                if g[0][0] == 0:  # first group of chunk -> new outT bank
                    outT_cur = ps.tile(
                        [HS + 1, 512], F32, tag="outT", bufs=1, name=f"oT{ic}"
                    )
                sg = ps.tile([128, 1536], F32, tag="sg", bufs=2, name=f"sg{ic}_{g[0][0]}")
                emit_scores(ic, g, sg)
                if pending is not None:
                    pic, pg, ppt, pouT, plast = pending
                    emit_pv(pic, pg, ppt, pouT, plast)
                    if plast:
                        emit_out(pic, pouT)
                pt = wpool.tile(
                    [128, 1536], BF16, tag="pt", bufs=4, name=f"pt{ic}_{g[0][0]}"
                )
                nc.scalar.activation(
                    pt[:, 0:fd], sg[:, 0:fd], AF.Exp, bias=0.0, scale=SCALE
                )
                pending = (ic, g, pt, outT_cur, last)
                for cc in bg:
                    proj_chunk(cc)

            pic, pg, ppt, pouT, plast = pending
            emit_pv(pic, pg, ppt, pouT, plast)
            emit_out(pic, pouT)

    import concourse.mybir as mybir
    _split_excess_waits(nc, mybir, limit=1)
    return nc


def _get_nc():
    if "nc" not in _CACHE:
        _CACHE["nc"] = _build_nc()
    return _CACHE["nc"]



def kernel(x_text_emb, Wq, Wk, Wv, freqs_cos, freqs_sin, x_latex_mask):
    import ml_dtypes
    from concourse.bass_utils import run_bass_kernel_spmd

    bf16 = ml_dtypes.bfloat16
    nc = _get_nc()

    swap = np.arange(HS) ^ 1
    cos2 = np.repeat(np.asarray(freqs_cos, np.float32).T, 2, axis=0)  # [64, T]
    sin2s = np.repeat(np.asarray(freqs_sin, np.float32).T, 2, axis=0)
    sin2s[0::2] *= -1.0
    cosx = np.ascontiguousarray(np.tile(cos2, (2, 1))).astype(bf16)  # [128, T]
    sinx = np.ascontiguousarray(np.tile(sin2s, (2, 1))).astype(bf16)
    Wq = np.asarray(Wq, np.float32)
    Wk = np.asarray(Wk, np.float32)
    Wv = np.asarray(Wv, np.float32)
    w = np.concatenate(
        [Wq, Wq, Wq[:, swap], Wq[:, swap], Wk, Wk, Wk[:, swap], Wk[:, swap], Wv],
        axis=1,
    ).astype(bf16)
    w = np.ascontiguousarray(w)
    jj, ii = np.meshgrid(np.arange(128), np.arange(128), indexing="ij")
    tri = np.concatenate(
        [np.eye(128, dtype=np.float32), np.where(ii < jj, -1e30, 0.0)], axis=1
    ).astype(bf16)
    tri = np.ascontiguousarray(tri)
    # mask01[b] laid out [j_in_tile(128), j_tile(NT)]
    mask01 = np.asarray(x_latex_mask != 0, np.float32).reshape(N_CORES, NT, 128)

    in_maps = []
    for b in range(N_CORES):
        in_maps.append(
            {
                "xT": np.ascontiguousarray(
                    np.asarray(x_text_emb[b], np.float32).T
                ).astype(bf16),
                "w": w,
                "cosx": cosx,
                "sinx": sinx,
                "mask01": np.ascontiguousarray(mask01[b].T),
                "tri": tri,
            }
        )

    res = run_bass_kernel_spmd(nc, in_maps, core_ids=list(range(N_CORES)))
    # out arrives [65, T]: rows 0-63 = outT[d, i], row 64 = rowsum[i]
    outs = []
    for b in range(N_CORES):
        r = np.asarray(res.results[b]["out"], np.float32)
        outs.append((r[0:HS, :] / r[HS : HS + 1, :]).T)
    return np.stack(outs, axis=0)
